# revision 62
# baseline (speedup 1.0000x reference)
"""Trainium2 Bass kernel for nn_ConvSPE (depthwise-conv SPE + per-channel contraction).

Math (reference): per bn=(b,nu) row and channel d:
    pe_k = noise / sqrt(num*d)                       (b*num, d, s+2k)
    pe_q = depthwise_valid_xcorr(pe_k, w)            k=200 taps, same filter per channel
    qhat[b,nu,t] = sum_d pe_q[bn,d,t]      * q[b,d,t]
    khat[b,nu,t] = sum_d pe_k[bn,d,t+k//2] * k[b,d,t]

Kernel strategy (8 NeuronCores, data-parallel over the 128 bn rows; 16 rows/core):
  * qhat conv: host pre-arranges noise into a time-partition-inner fp16 layout
    xt[p, n, d] = noise[bn, d, 128n+p]; the conv becomes 3 PSUM-accumulated
    TensorE matmuls per output half with fixed Toeplitz weights
    W_s[p, m] = w[p + 128s - m]. ACT drains PSUM; DVE multiplies by
    host-pre-transposed queries (fp16 2x); d-reduce via fp16 add trees.
  * khat is split by time range to balance all five engines:
      - u-blocks 0..16 run ENTIRELY on the TensorE as block-diagonal
        keys-as-weights matmuls: contraction (d-half, 4 time positions) =
        128 partitions, po = 4 time positions, free = 16 bn rows, two
        d-half passes accumulated in PSUM; ACT drains the [4, 512] tiles.
        Needs a second host layout of noise (d on partitions) for that
        range only — the extra DMA fits in the DMA device's slack.
      - u-blocks 17..32 go the elementwise way: DVE products (fp16 2x),
        GpSimd add trees.
    This removes ~half of the khat products+reduces from DVE/GpSimd,
    whose combined load was the previous critical path.
"""

import math
import numpy as np

_CACHE = {}


def _ensure_paths():
    try:
        import concourse  # noqa: F401
    except ImportError:
        import sys

        for p in ("/opt/trn_rl_repo", "/root/.axon_site/_ro/trn_rl_repo"):
            if p not in sys.path:
                sys.path.insert(0, p)


N_CORES = 8
B, D, L, K, NUM = 4, 64, 4096, 200, 32
NW = 34  # x windows of 128 loaded per row (covers t+j up to 4351)
NT = 32  # output time blocks of 128
NK = 33  # khat blocks (u = t + 100 spans [0, 4224))
ROWS = 16  # bn rows per core
RB = 4  # rows per reduce batch
NKP = 17  # khat u-blocks computed on the PE (u in [0, 2176))
NKE = NK - NKP  # khat u-blocks via DVE/GpSimd elementwise (16)
NGP = NKP * 32  # PE-side 4-wide time groups (544)
# xd chunking: one chunk per conv batch, sized in whole 32-group psum blocks
KCH_BLKS = (4, 4, 4, 5)

# qhat reduce batches (4 rows each) on GpSimd instead of DVE.
QHAT_POOL_BATCHES = frozenset()


def _add_tree(eng, a, b, acc_out, src):
    """Reduce src [128, rb, n, 64] over the last axis into acc_out
    [128, rb, n] with a 6-level fp16 add tree (DVE 2x mode)."""
    eng.tensor_add(a[:], src[:, :, :, 0:32], src[:, :, :, 32:64])
    eng.tensor_add(b[:], a[:, :, :, 0:16], a[:, :, :, 16:32])
    eng.tensor_add(a[:, :, :, 0:8], b[:, :, :, 0:8], b[:, :, :, 8:16])
    eng.tensor_add(b[:, :, :, 0:4], a[:, :, :, 0:4], a[:, :, :, 4:8])
    eng.tensor_add(a[:, :, :, 8:10], b[:, :, :, 0:2], b[:, :, :, 2:4])
    eng.tensor_add(acc_out, a[:, :, :, 8], a[:, :, :, 9])


def build_module():
    """Build + compile the per-core Bass module (identical SPMD program)."""
    _ensure_paths()
    from contextlib import ExitStack

    import concourse.bacc as bacc
    import concourse.mybir as mybir
    import concourse.tile as tile

    F16 = mybir.dt.float16
    F32 = mybir.dt.float32

    nc = bacc.Bacc(
        "TRN2", target_bir_lowering=False, debug=False, num_devices=N_CORES
    )

    xf_d = nc.dram_tensor("xf", [ROWS, 128, NW, D], F16, kind="ExternalInput").ap()
    wq_d = nc.dram_tensor("wq", [3, 128, 128], F16, kind="ExternalInput").ap()
    qt_d = nc.dram_tensor("qt", [128, NT, D], F16, kind="ExternalInput").ap()
    # elementwise khat keys (blocks 17..32): kf[p, nn, d]
    kf_d = nc.dram_tensor("kf", [128, NKE, D], F16, kind="ExternalInput").ap()
    # PE khat stream: xd[4*dlo+t2, pi, g, bn] = noise[bn, 32pi+dlo, 4g+t2]
    xd_d = nc.dram_tensor("xd", [128, 2, NGP, ROWS], F16, kind="ExternalInput").ap()
    # PE khat weights: kw[4*dlo+t2, pi, g, po] = kfm[32pi+dlo, 4g+po]*(po==t2)
    kw_d = nc.dram_tensor("kw", [128, 2, NGP, 4], F16, kind="ExternalInput").ap()
    qo_d = nc.dram_tensor("qo", [128, ROWS, NT], F32, kind="ExternalOutput").ap()
    kpo_d = nc.dram_tensor("kpo", [4, NKP, 32 * ROWS], F16, kind="ExternalOutput").ap()
    keo_d = nc.dram_tensor("keo", [128, ROWS, NKE], F32, kind="ExternalOutput").ap()

    NB = ROWS // RB

    with tile.TileContext(nc) as tc, ExitStack() as ctx:
        wp = ctx.enter_context(tc.tile_pool(name="const", bufs=1))
        xp = ctx.enter_context(tc.tile_pool(name="x", bufs=4))
        xdp = ctx.enter_context(tc.tile_pool(name="xd", bufs=2))
        pp = ctx.enter_context(tc.tile_pool(name="psum", bufs=3, space="PSUM"))
        kp = ctx.enter_context(tc.tile_pool(name="kpsum", bufs=2, space="PSUM"))
        cp = ctx.enter_context(tc.tile_pool(name="peq", bufs=3))
        qp = ctx.enter_context(tc.tile_pool(name="prodq", bufs=3))
        kpool = ctx.enter_context(tc.tile_pool(name="prodk", bufs=3))
        tp = ctx.enter_context(tc.tile_pool(name="tree", bufs=2))
        op = ctx.enter_context(tc.tile_pool(name="out", bufs=1))

        # constants ride the ACT HWDGE queue, x streams ride SP: the two
        # queues' DGE latencies overlap so xt0/kf land sooner
        wts = []
        for s in range(3):
            t = wp.tile([128, 128], F16, tag=f"w{s}")
            nc.scalar.dma_start(t[:], wq_d[s])
            wts.append(t)
        xts = []
        xt = xp.tile([128, NW, D], F16, tag="xt", name="xt_0")
        nc.sync.dma_start(xt[:, 0:12, :], xf_d[0, :, 0:12, :])
        nc.sync.dma_start(xt[:, 12:NW, :], xf_d[0, :, 12:NW, :])
        xts.append(xt)
        kf_t = wp.tile([128, NKE, D], F16, tag="kf")
        nc.sync.dma_start(kf_t[:], kf_d[:])
        xt = xp.tile([128, NW, D], F16, tag="xt", name="xt_1")
        nc.sync.dma_start(xt[:], xf_d[1])
        xts.append(xt)
        # batch 0's remaining rows go before qt/kw/xd so the khat-elem muls
        # (which gate Pool's first tree) aren't stuck behind them
        for bn in range(2, RB):
            xt = xp.tile([128, NW, D], F16, tag="xt", name=f"xt_{bn}")
            nc.sync.dma_start(xt[:], xf_d[bn])
            xts.append(xt)
        qt_t = wp.tile([128, NT, D], F16, tag="qt")
        nc.scalar.dma_start(qt_t[:], qt_d[:])
        kw_t = wp.tile([128, 2, NGP, 4], F16, tag="kw")
        nc.scalar.dma_start(kw_t[:], kw_d[:])

        qacc = op.tile([128, ROWS, NT], F32, tag="qa")
        kacc_p = op.tile([4, NKP, 32 * ROWS], F16, tag="kap")
        kacc_e = op.tile([128, ROWS, NKE], F32, tag="kae")

        # xd chunk tiles, DMA'd one conv batch ahead of use
        chunk_base = [sum(KCH_BLKS[:i]) for i in range(NB)]  # in psum blocks
        xds = []

        def xd_load(c):
            nblks = KCH_BLKS[c]
            g0 = chunk_base[c] * 32
            xdt = xdp.tile(
                [128, 2, nblks * 32, ROWS], F16, tag="xd", name=f"xd_{c}"
            )
            nc.sync.dma_start(xdt[:], xd_d[:, :, g0 : g0 + nblks * 32, :])
            xds.append(xdt)

        def khat_chunk(c):
            """Block-diag matmuls for xd chunk c (KCH_BLKS[c] psum blocks)."""
            xdt = xds[c]
            for j in range(KCH_BLKS[c]):
                nblk = chunk_base[c] + j
                kps = kp.tile([4, 32 * ROWS], F32, tag="kps", name=f"kps_{nblk}")
                for gm in range(32):
                    g = nblk * 32 + gm
                    for pi in range(2):
                        nc.tensor.matmul(
                            kps[:, gm * ROWS : (gm + 1) * ROWS],
                            kw_t[:, pi, g, :],
                            xdt[:, pi, j * 32 + gm, :],
                            start=(pi == 0),
                            stop=(pi == 1),
                        )
                nc.scalar.copy(kacc_p[:, nblk, :], kps[:])

        xd_load(0)
        pending_prev = []
        pending_cur = []

        for rb in range(NB):
            pqb = qp.tile([128, RB, NT, D], F16, tag="pq", name=f"pq_{rb}")
            pkb = kpool.tile([128, RB, NKE, D], F16, tag="pk", name=f"pk_{rb}")

            # khat elementwise products first: they gate Pool's trees
            for r in range(RB):
                bn = rb * RB + r
                if bn >= RB:
                    xt = xp.tile([128, NW, D], F16, tag="xt", name=f"xt_{bn}")
                    nc.sync.dma_start(xt[:], xf_d[bn])
                    xts.append(xt)
                nc.vector.tensor_mul(
                    pkb[:, r], xts[bn][:, NKP : NKP + NKE, :], kf_t[:]
                )
            if rb > 0:
                xd_load(rb)

            # GpSimd reduces this batch's elementwise khat products
            ka = tp.tile([128, RB, NKE, 32], F16, tag="ka", name=f"ktA_{rb}")
            kb = tp.tile([128, RB, NKE, 16], F16, tag="kb", name=f"ktB_{rb}")
            _add_tree(nc.gpsimd, ka, kb, kacc_e[:, rb * RB : (rb + 1) * RB, :], pkb)

            for r in range(RB):
                bn = rb * RB + r
                xt = xts[bn]
                # conv: 3 Toeplitz matmuls per 8-block group, 2 PSUM halves
                # per row so ACT drains half 0 while PE accumulates half 1
                for h in range(2):
                    ps = pp.tile(
                        [128, NT // 2, D], F32, tag="ps", name=f"ps_{bn}_{h}"
                    )
                    for s in range(3):
                        for g in range(2 * h, 2 * h + 2):
                            nc.tensor.matmul(
                                ps[:, (g - 2 * h) * 8 : (g - 2 * h + 1) * 8, :],
                                wts[s][:],
                                xt[:, g * 8 + s : g * 8 + s + 8, :],
                                start=(s == 0),
                                stop=(s == 2),
                            )
                    peq = cp.tile(
                        [128, NT // 2, D], F16, tag="peq", name=f"peq_{bn}_{h}"
                    )
                    nc.scalar.copy(peq[:], ps[:])
                    nc.vector.tensor_mul(
                        pqb[:, r, h * (NT // 2) : (h + 1) * (NT // 2), :],
                        peq[:],
                        qt_t[:, h * (NT // 2) : (h + 1) * (NT // 2), :],
                    )

            # PE khat chunk slots in behind this batch's conv matmuls
            khat_chunk(rb)

            # ---- batched qhat reduce tree; last batch split in two for a
            # shorter tail (second half only waits on rows 14-15)
            halves = (
                [(0, RB)]
                if rb < NB - 1
                else [(0, RB // 2), (RB // 2, RB)]
            )
            for h0, h1 in halves:
                qa = tp.tile(
                    [128, h1 - h0, NT, 32], F16, tag="qa", name=f"qtA_{rb}_{h0}"
                )
                qb = tp.tile(
                    [128, h1 - h0, NT, 16], F16, tag="qb", name=f"qtB_{rb}_{h0}"
                )
                qout = qacc[:, rb * RB + h0 : rb * RB + h1, :]
                pqs = pqb[:, h0:h1]
                if rb in QHAT_POOL_BATCHES:
                    _add_tree(nc.gpsimd, qa, qb, qout, pqs)
                else:
                    pending_cur.append(
                        lambda qa=qa, qb=qb, qout=qout, pqs=pqs: _add_tree(
                            nc.vector, qa, qb, qout, pqs
                        )
                    )

            # emit the previous batch's DVE trees behind this batch's muls
            for job in pending_prev:
                job()
            pending_prev = pending_cur
            pending_cur = []

        # khat outputs and finished qhat rows ship before the final trees
        nc.sync.dma_start(kpo_d[:], kacc_p[:])
        nc.sync.dma_start(keo_d[:], kacc_e[:])
        nc.sync.dma_start(qo_d[:, 0:12, :], qacc[:, 0:12, :])

        for job in pending_prev:
            job()

        nc.sync.dma_start(qo_d[:, 12:ROWS, :], qacc[:, 12:ROWS, :])

    nc.compile()
    return nc


def _get_module():
    if "nc" not in _CACHE:
        _CACHE["nc"] = build_module()
    return _CACHE["nc"]


def make_in_maps(queries, keys, noise, conv_weight, num):
    """Host-side shard + re-layout (all cheap numpy ops)."""
    num = int(np.asarray(num))
    queries = np.asarray(queries, dtype=np.float32)
    keys = np.asarray(keys, dtype=np.float32)
    noise = np.asarray(noise, dtype=np.float32)
    w = np.asarray(conv_weight, dtype=np.float32)[0, 0, :]
    scale = 1.0 / math.sqrt(num * D)

    # Toeplitz weights (scale folded in): W_s[p, m] = w[p + 128s - m] * scale
    p = np.arange(128)[:, None]
    m = np.arange(128)[None, :]
    Wq = np.zeros((3, 128, 128), np.float32)
    for s in range(3):
        j = p + 128 * s - m
        mask = (j >= 0) & (j < K)
        Wq[s][mask] = w[j[mask]] * scale
    Wq16 = Wq.astype(np.float16)

    # xf[bn][p, n, d] = noise[bn, d, 128n + p]
    xf = (
        noise[:, :, : NW * 128]
        .reshape(B * NUM, D, NW, 128)
        .transpose(0, 3, 2, 1)
        .astype(np.float16)
    )
    # qt[b][p, tau, d] = queries[b, d, 128 tau + p]
    qt = queries.reshape(B, D, NT, 128).transpose(0, 3, 2, 1).astype(np.float16)

    # khat factors over u = t + 100 in [0, 4224):
    # kfm[b][d, u] = keys[b, d, u - 100] * scale (zero out of range)
    kfm = np.zeros((B, D, NK * 128), np.float32)
    kfm[:, :, K // 2 : K // 2 + L] = keys * scale

    # elementwise part (u-blocks NKP..NK): kf[b][p, nn, d]
    kf = (
        kfm[:, :, NKP * 128 :]
        .reshape(B, D, NKE, 128)
        .transpose(0, 3, 2, 1)
        .astype(np.float16)
    )

    # PE part (u < NKP*128): xd[4*dlo+t2, pi, g, bn], kw block-diag
    xm = noise[:, :, : NGP * 4]
    xd_all = (
        xm.reshape(B * NUM, 2, 32, NGP, 4)  # [bn, pi, dlo, g, t2]
        .transpose(2, 4, 1, 3, 0)  # [dlo, t2, pi, g, bn]
        .astype(np.float16)
    )
    kfg = (
        kfm[:, :, : NGP * 4].reshape(B, 2, 32, NGP, 4).astype(np.float16)
    )  # [b, pi, dlo, g, po]
    eye4 = np.eye(4, dtype=np.float16)  # [t2, po]
    kw = np.einsum("bpdgc,tc->bdtpgc", kfg, eye4)
    kw = kw.reshape(B, 128, 2, NGP, 4)

    in_maps = []
    for c in range(N_CORES):
        b = c // 2
        rows = slice(ROWS * c, ROWS * (c + 1))
        xd_core = xd_all[:, :, :, :, rows].reshape(128, 2, NGP, ROWS)
        in_maps.append(
            {
                "xf": np.ascontiguousarray(xf[rows]),
                "wq": Wq16,
                "qt": np.ascontiguousarray(qt[b]),
                "kf": np.ascontiguousarray(kf[b]),
                "xd": np.ascontiguousarray(xd_core),
                "kw": np.ascontiguousarray(kw[b]),
            }
        )
    return in_maps


def assemble_outputs(results):
    qhat = np.empty((B * NUM, L), np.float32)
    khat = np.empty((B * NUM, L), np.float32)
    for c in range(N_CORES):
        qo = results[c]["qo"]  # [128, ROWS, NT]
        kpo = results[c]["kpo"]  # [4, NKP, 32*ROWS]
        keo = results[c]["keo"]  # [128, ROWS, NKE]
        qhat[ROWS * c : ROWS * (c + 1)] = qo.transpose(1, 2, 0).reshape(ROWS, L)
        # kpo[t2, n, 16*gm + bn] = khat_u[bn, 128n + 4gm + t2]
        kv = np.empty((ROWS, NK * 128), np.float32)
        kv[:, : NKP * 128] = (
            kpo.reshape(4, NKP, 32, ROWS)
            .transpose(3, 1, 2, 0)  # [bn, n, gm, t2]
            .reshape(ROWS, NKP * 128)
        )
        kv[:, NKP * 128 :] = keo.transpose(1, 2, 0).reshape(ROWS, NKE * 128)
        khat[ROWS * c : ROWS * (c + 1)] = kv[:, K // 2 : K // 2 + L]
    return (
        qhat.reshape(B, NUM, L),
        khat.reshape(B, NUM, L),
    )


def kernel(queries, keys, noise, conv_weight, num):
    _ensure_paths()
    from concourse import bass_utils

    in_maps = make_in_maps(queries, keys, noise, conv_weight, num)
    nc = _get_module()
    res = bass_utils.run_bass_kernel_spmd(nc, in_maps, core_ids=list(range(N_CORES)))
    return assemble_outputs(res.results)
